# revision 1
# baseline (speedup 1.0000x reference)
"""Tensor-parallel causal self-attention (GQA + RoPE) on one TRN2 chip (8 NeuronCores).

Megatron-style TP over heads: core i computes q-heads {2i, 2i+1} (kv head i//2),
runs blocked causal attention for those heads entirely on-core, then the partial
c_proj  y_i @ Wo[rows_i, :].  The 8 partial [T, C] outputs are summed on the host
(the TP all-reduce), which is pure gather/unshard data movement.

Layout strategy (everything transposed so the contraction dim sits on SBUF
partitions):
  xT   [C, T]   (host pre-transposed, bf16)
  qT/kT = Wq/Wk-proj emitted directly as [HD, T] via lhsT=W, rhs=xT
  RoPE rotate_half runs on PE as a +-1 permutation matmul (DVE cannot read two
      SBUF operands at different base partitions)
  scoresT [s,t] = kT_tile.T @ qT  -> exp on ACT; softmax denominator = per-tile
      ones.T @ p matmuls accumulating in PSUM (keeps the serial RMW chain off
      Pool); normalization folded into the PSUM->SBUF eviction multiply
  v natural [s,d] obtained from a vT projection + DMA-transpose (N=512 matmuls
      instead of N=128)
  outT [d,t] += v_tile.T-style accumulation with lhsT=v_nat, rhs=pT
  c_proj: lhsT=yT slices, rhs=Wo rows -> natural [t,c] psum -> full-row SBUF
      staging -> one 1MB DMA per 128-row block.
Causal masking: off-diagonal s-tiles need no mask; the 4 diagonal s-tiles per
t-block are computed at narrowed width (only columns t >= 128j) with a -1e30
additive mask for the intra-tile triangle.
"""

import math
from contextlib import ExitStack

import ml_dtypes
import numpy as np

import concourse.bass as bass
import concourse.tile as tile
from concourse import bacc, mybir
from concourse.bass import ts, ds
from concourse.bass_utils import run_bass_kernel_spmd

# ---------------- problem constants (hardcoded per contest rules) ------------
B, T, C = 1, 2048, 2048
H, KH, HD = 16, 4, 128
NCORES = 8
HQ = H // NCORES            # 2 query heads per core
ROPE_BASE = 10000.0
SCALE = 1.0 / math.sqrt(HD)
TB = 512                    # t-block (moving free dim) for attention
NT = T // TB                # 4
NCT = C // 128              # 16 contraction tiles for projections
NS = T // 128               # 16 key/value s-tiles
BF16 = mybir.dt.bfloat16
F32 = mybir.dt.float32
EXPF = mybir.ActivationFunctionType.Exp
NEG = -1.0e30

_NC_CACHE = {}


def _bf16(a):
    return np.ascontiguousarray(np.asarray(a, dtype=np.float32).astype(ml_dtypes.bfloat16))


def _emit(tc, dr, out_d):
    nc = tc.nc
    with ExitStack() as ctx:
        def sb(name, bufs):
            return ctx.enter_context(tc.tile_pool(name=name, bufs=bufs))

        def ps(name, bufs):
            return ctx.enter_context(tc.tile_pool(name=name, bufs=bufs, space="PSUM"))

        p_xt = sb("xt", NCT)
        p_wq = sb("wq", NCT)
        p_wk = sb("wk", NCT)
        p_wv = sb("wv", NCT)
        p_wo = sb("wo", HQ)
        p_trig = sb("trig", 2)
        p_mask = sb("mask", 1)
        p_ones = sb("ones", 1)
        p_qt = sb("qt", HQ)
        p_kt = sb("kt", 1)
        p_v = sb("v", NS)
        p_vt = sb("vt", 2)
        p_yt = sb("yt", HQ)
        p_qraw = sb("qraw", 3)
        p_rtmp = sb("rtmp", 6)
        p_pt = sb("pt", 8)
        p_r = sb("r", 3)
        p_rb = sb("rb", 3)
        p_stage = sb("stage", 4)
        ps_a = ps("ps_a", 2)      # qkv projection chains + c_proj chains
        ps_sc = ps("ps_sc", 3)    # rope-rot + score tiles
        ps_o = ps("ps_o", 2)      # attention outT accumulation chains
        ps_dn = ps("ps_dn", 1)    # packed denominator pairs [33, TB]

        # ---------------- input loads (spread across issue queues) -----------
        wqkv = [p_wq.tile([128, 4 * HD], BF16, name=f"wqkv{i}", tag="wq")
                for i in range(NCT)]
        for i in range(NCT):
            nc.scalar.dma_start(wqkv[i][:], dr["wqkv"][ts(i, 128), :])
        wq = wqkv                                    # cols [0, 256) = Wq
        wk = [t[:, 256:384] for t in wqkv]           # cols [256, 384) = Wk
        wv = [t[:, 384:512] for t in wqkv]           # cols [384, 512) = Wv
        xt = [p_xt.tile([128, T], BF16, name=f"xt{i}", tag="xt") for i in range(NCT)]
        # small leading slice so the 8-chain group's first matmuls (t-blocks
        # 0+1) unblock ~1us sooner; rest of tile 0 follows, then full tiles.
        nc.sync.dma_start(xt[0][:, 0:1024], dr["xt"][0:128, 0:1024])
        nc.sync.dma_start(xt[0][:, 1024:T], dr["xt"][0:128, 1024:T])
        for i in range(1, NCT):
            nc.sync.dma_start(xt[i][:], dr["xt"][ts(i, 128), :])
        wo2 = p_wo.tile([128, HQ * C], BF16, name="wo2", tag="wo")
        wo = [wo2[:, ts(h, C)] for h in range(HQ)]
        trig = p_trig.tile([128, 2 * T], BF16, name="trig", tag="trig")
        nc.sync.dma_start(trig[:], dr["trig"][:, :])
        cost = trig[:, 0:T]
        sint = trig[:, T:2 * T]
        nc.sync.dma_start(wo2[:], dr["wo"][:, :])
        masks = p_mask.tile([128, 128], F32, name="masks", tag="mask")
        nc.gpsimd.dma_start(masks[:], dr["masks"][:, :])
        rmat = p_ones.tile([128, 128], BF16, name="rmat", tag="rmat")
        nc.gpsimd.dma_start(rmat[:], dr["rmat"][:, :])
        ones = p_ones.tile([128, 1], BF16, name="ones", tag="ones")
        nc.vector.memset(ones[:], 1.0)

        # ---------------- helpers ----------------
        def rope_evict(psum, bt, dst, rot_pool=None):
            """psum [128(d), TB] f32 -> RoPE -> dst (bf16 slice [128, TB])."""
            rp = rot_pool if rot_pool is not None else ps_sc
            cs = cost[:, ts(bt, TB)]
            sn = sint[:, ts(bt, TB)]
            raw = p_qraw.tile([128, TB], BF16, name="rraw", tag="qraw")
            nc.scalar.copy(raw[:], psum[:])
            rot = rp.tile([128, TB], F32, name="rot", tag=rp.name)
            nc.tensor.matmul(rot[:], lhsT=rmat[:], rhs=raw[:], start=True, stop=True)
            t1 = p_rtmp.tile([128, TB], BF16, name="rt1", tag="rtmp")
            t2 = p_rtmp.tile([128, TB], BF16, name="rt2", tag="rtmp")
            nc.vector.tensor_mul(t1[:], raw[:], cs)
            nc.vector.tensor_mul(t2[:], rot[:], sn)
            nc.gpsimd.tensor_add(dst, t1[:], t2[:])

        # -------- per t-block: projections -> attention -> c_proj ------------
        chain_n = [0]

        def proj_psum():
            pool = (ps_a, ps_o)[chain_n[0] % 2]
            chain_n[0] += 1
            return pool.tile([128, TB], F32, name="pp", tag=pool.name)

        qT = [p_qt.tile([128, T], BF16, name=f"qT{h}", tag="qt") for h in range(HQ)]
        kT = p_kt.tile([128, T], BF16, name="kT", tag="kt")
        v = [p_v.tile([128, HD], BF16, name=f"v{s}", tag="v") for s in range(NS)]
        yT = [p_yt.tile([128, T], BF16, name=f"yT{h}", tag="yt") for h in range(HQ)]
        dma_rr = [nc.sync, nc.scalar, nc.gpsimd]

        def v_evict(pv, bt):
            vts = p_vt.tile([128, TB], BF16, name="vts", tag="vt")
            nc.scalar.copy(vts[:], pv[:])
            for j in range(TB // 128):
                nc.sync.dma_start_transpose(v[4 * bt + j][:], vts[:, ts(j, 128)])

        def proj_block(bt, psum_of=None, pools_of=None):
            """Emit the 4 projection chains + evictions for t-block bt.
            psum_of: optional list of 4 pre-allocated psum tiles (grouped,
            ci-interleaved emission); default = chain-wise with 2-pool RR."""
            if psum_of is None:
                tiles = []
                pools = []
                specs = [(wq, 0), (wq, 1), (wk, None), (wv, None)]
                for kind in range(4):
                    pools.append((ps_a, ps_o)[chain_n[0] % 2])
                    p = proj_psum()
                    tiles.append(p)
                    w, h = specs[kind]
                    for ci in range(NCT):
                        lhsT = w[ci][:, ts(h, HD)] if h is not None else w[ci]
                        nc.tensor.matmul(
                            p[:], lhsT=lhsT, rhs=xt[ci][:, ts(bt, TB)],
                            start=(ci == 0), stop=(ci == NCT - 1))
            else:
                tiles = psum_of
                pools = pools_of
            rope_evict(tiles[0], bt, qT[0][:, ts(bt, TB)])
            rope_evict(tiles[1], bt, qT[1][:, ts(bt, TB)])
            rope_evict(tiles[2], bt, kT[:, ts(bt, TB)])
            v_evict(tiles[3], bt)

        # t-blocks 0+1: one ci-interleaved 8-chain group across all 8 PSUM
        # banks — maximizes PE progress while x is still streaming in.
        grp_pools = [ps_a, ps_o, ps_sc, ps_dn, ps_a, ps_o, ps_sc, ps_sc]
        grp = [pl.tile([128, TB], F32, name=f"gp{i}", tag=pl.name)
               for i, pl in enumerate(grp_pools)]
        for ci in range(NCT):
            for i in range(8):
                b = i // 4
                kind = i % 4
                w, h = [(wq, 0), (wq, 1), (wk, None), (wv, None)][kind]
                lhsT = w[ci][:, ts(h, HD)] if h is not None else w[ci]
                nc.tensor.matmul(
                    grp[i][:], lhsT=lhsT, rhs=xt[ci][:, ts(b, TB)],
                    start=(ci == 0), stop=(ci == NCT - 1))
        proj_block(0, psum_of=grp[0:4], pools_of=grp_pools[0:4])
        proj_block(1, psum_of=grp[4:8], pools_of=grp_pools[4:8])

        # ---------------- attention + c_proj, pipelined per t-block ----------
        def attn_block(bt):
            dnp = ps_dn.tile([33, TB], F32, name="dnp", tag="ps_dn")
            nbs = 4 * (bt + 1)
            for h in range(HQ):
                po = ps_o.tile([128, TB], F32, name="po", tag="ps_o")
                dn = dnp[32 * h:32 * h + 1, :]
                for bs in range(nbs):
                    j = bs - 4 * bt
                    off = max(j, 0) * 128        # first live column of this tile
                    w = TB - off
                    sc = ps_sc.tile([128, TB], F32, name="sc", tag="ps_sc")
                    nc.tensor.matmul(
                        sc[:, off:TB], lhsT=kT[:, ts(bs, 128)],
                        rhs=qT[h][:, ds(bt * TB + off, w)],
                        start=True, stop=True)
                    if j >= 0:  # diagonal block: intra-tile causal triangle
                        nc.vector.tensor_add(
                            sc[:, off:off + 128], sc[:, off:off + 128],
                            masks[:, :])
                    pt = p_pt.tile([128, TB], BF16, name="pt", tag="pt")
                    nc.scalar.activation(pt[:, off:TB], sc[:, off:TB], EXPF,
                                         scale=SCALE)
                    nc.tensor.matmul(
                        dn[:, off:TB], lhsT=ones[:], rhs=pt[:, off:TB],
                        start=(bs == 0), stop=(bs == nbs - 1))
                    nc.tensor.matmul(
                        po[:, off:TB], lhsT=v[bs][:], rhs=pt[:, off:TB],
                        start=(bs == 0), stop=(bs == nbs - 1))
                r = p_r.tile([1, TB], F32, name="r", tag="r")
                nc.vector.reciprocal(r[:], dn[:])
                rb = p_rb.tile([128, TB], F32, name="rb", tag="rb")
                nc.gpsimd.partition_broadcast(rb[:], r[:])
                for e in range(4):   # chunked evict: c_proj(m) starts earlier
                    nc.vector.tensor_mul(
                        yT[h][:, ds(bt * TB + e * 128, 128)],
                        po[:, ts(e, 128)], rb[:, ts(e, 128)])
            # c_proj rows for this t-block (both heads now final)
            for sub in range(TB // 128):
                m = (TB // 128) * bt + sub
                st = p_stage.tile([128, C], BF16, name="st", tag="stage")
                for n in range(C // TB):
                    pc = ps_a.tile([128, TB], F32, name="pc", tag="ps_a")
                    for h in range(HQ):
                        nc.tensor.matmul(
                            pc[:], lhsT=yT[h][:, ts(m, 128)], rhs=wo[h][:, ts(n, TB)],
                            start=(h == 0), stop=(h == HQ - 1))
                    if n % 2 == 0:
                        nc.scalar.copy(st[:, ts(n, TB)], pc[:])
                    else:
                        nc.vector.tensor_copy(st[:, ts(n, TB)], pc[:])
                    if m == NS - 1:   # last row-block: chunked DMA, short tail
                        nc.sync.dma_start(out_d[ts(m, 128), ts(n, TB)],
                                          st[:, ts(n, TB)])
                if m < NS - 1:
                    dma_rr[m % 3].dma_start(out_d[ts(m, 128), :], st[:])

        proj_block(2)
        proj_block(3)
        attn_block(0)
        attn_block(1)
        attn_block(2)
        attn_block(3)


def build_nc():
    if "nc" in _NC_CACHE:
        return _NC_CACHE["nc"]
    nc = bacc.Bacc("TRN2", target_bir_lowering=False, debug=False, num_devices=NCORES)
    dr = {}

    def din(name, shape, dt):
        dr[name] = nc.dram_tensor(name, shape, dt, kind="ExternalInput").ap()

    din("xt", (C, T), BF16)
    din("wqkv", (C, 4 * HD), BF16)
    din("wo", (HD, HQ * C), BF16)
    din("trig", (HD, 2 * T), BF16)
    din("masks", (128, 128), F32)
    din("rmat", (HD, HD), BF16)
    out_d = nc.dram_tensor("out", (T, C), BF16, kind="ExternalOutput").ap()

    with tile.TileContext(nc) as tc:
        _emit(tc, dr, out_d)
    nc.compile()
    _NC_CACHE["nc"] = nc
    return nc


def make_in_maps(x, Wq, Wk, Wv, Wo, position_ids):
    """Host-side sharding + constant tables. Returns one input dict per core."""
    x = np.asarray(x, dtype=np.float32)
    xt = _bf16(x.reshape(T, C).T)                      # [C, T]

    pos = np.asarray(position_ids).astype(np.float64)  # [T]
    inv = 1.0 / (ROPE_BASE ** (np.arange(0, HD, 2, dtype=np.float64) / HD))
    fr = pos[:, None] * inv[None, :]                   # [T, 64]
    emb = np.concatenate([fr, fr], axis=-1)            # [T, 128]
    cost = _bf16(np.cos(emb).T)                        # [128, T]
    sint = _bf16(np.sin(emb).T)

    si = np.arange(128)[:, None]
    ti = np.arange(128)[None, :]
    masks = np.where(si > ti, NEG, 0.0).astype(np.float32)   # [128, 128] triangle

    # rotate_half operator: rot = R @ q  with  rot[d<64] = -q[d+64],
    # rot[d>=64] = q[d-64].  matmul computes lhsT.T @ rhs, so ship R.T.
    R = np.zeros((HD, HD), dtype=np.float32)
    R[np.arange(64), np.arange(64) + 64] = -1.0
    R[np.arange(64, 128), np.arange(64, 128) - 64] = 1.0
    rmat = _bf16(R.T)

    Wq = np.asarray(Wq, dtype=np.float32)
    Wk = np.asarray(Wk, dtype=np.float32)
    Wv = np.asarray(Wv, dtype=np.float32)
    Wo = np.asarray(Wo, dtype=np.float32)

    in_maps = []
    for i in range(NCORES):
        g = i // (NCORES // KH)                        # kv head for this core
        in_maps.append({
            "xt": xt,
            "wqkv": _bf16(np.concatenate([
                Wq[:, i * HQ * HD:(i + 1) * HQ * HD],
                Wk[:, g * HD:(g + 1) * HD],
                Wv[:, g * HD:(g + 1) * HD]], axis=1)),
            "wo": _bf16(np.concatenate([
                Wo[i * HQ * HD + h * HD:i * HQ * HD + (h + 1) * HD, :]
                for h in range(HQ)], axis=1)),
            "trig": np.concatenate([cost, sint], axis=1),
            "masks": masks,
            "rmat": rmat,
        })
    return in_maps


def run(inputs, trace=False):
    nc = build_nc()
    in_maps = make_in_maps(**inputs)
    res = run_bass_kernel_spmd(
        nc, in_maps, core_ids=list(range(NCORES)), trace=trace)
    out = np.zeros((T, C), dtype=np.float32)
    for i in range(NCORES):
        out += np.asarray(res.results[i]["out"], dtype=np.float32)
    return out.reshape(B, T, C), res


def kernel(x, Wq, Wk, Wv, Wo, position_ids):
    out, _ = run(dict(x=x, Wq=Wq, Wk=Wk, Wv=Wv, Wo=Wo,
                      position_ids=position_ids), trace=False)
    return out



# revision 30
# speedup vs baseline: 1.2126x; 1.2126x over previous
"""Tensor-parallel causal self-attention (GQA + RoPE) on one TRN2 chip (8 NeuronCores).

Megatron-style TP over heads: core i computes q-heads {2i, 2i+1} (kv head i//2),
runs blocked causal attention for those heads entirely on-core, then the partial
c_proj  y_i @ Wo[rows_i, :].  The 8 partial [T, C] outputs are summed on the host
(the TP all-reduce), which is pure gather/unshard data movement.

v2 layout strategy (everything transposed so the contraction dim sits on SBUF
partitions):
  xT   [C, T]   (host pre-transposed, bf16)
  qT/kT = Wq/Wk-proj emitted directly as [HD, T] via lhsT=W, rhs=xT
  RoPE rotate_half runs on PE as a +-1 permutation matmul (DVE cannot read two
      SBUF operands at different base partitions); split into raw-copy phase
      (ACT) and rot-matmul phase (PE) so chain PSUM frees early.
  scoresT [s,t]: two 128-s-tiles per [128,1024] PSUM strip (2 banks); ONE wide
      exp per strip on ACT (amortizes the ~352-cycle activation overhead).
  softmax denominator: DVE accumulates pt strips elementwise in bf16 (partition-
      lane partial sums), then one gpsimd partition_all_reduce per (h,t-block)
      produces the broadcast denominator; normalization folded into the
      PSUM->SBUF eviction multiply. No PE cycles spent on the denominator.
  v natural [s,d] obtained from a vT projection + DMA-transpose (N=512 matmuls
      instead of N=128)
  outT [d,t] += v_tile.T-style accumulation with lhsT=v_nat, rhs=pt
  c_proj: lhsT=yT slices, rhs=Wo rows -> natural [t,c] psum -> full-row SBUF
      staging -> one 1MB DMA per 128-row block; (m,n) items drained from a
      work-queue interleaved into later attention blocks to fill PE stalls
      while ACT runs exp.
Causal masking: off-diagonal s-tiles need no mask; the 4 diagonal s-tiles per
t-block are computed at narrowed width (only columns t >= 128j) with a -1e30
additive mask for the intra-tile triangle.  Dead strip regions are exp'd but
never read.
"""

import math
from contextlib import ExitStack

import ml_dtypes
import numpy as np

import concourse.bass as bass
import concourse.tile as tile
from concourse import bacc, mybir, bass_isa
from concourse.bass import ts, ds
from concourse.bass_utils import run_bass_kernel_spmd

# ---------------- problem constants (hardcoded per contest rules) ------------
B, T, C = 1, 2048, 2048
H, KH, HD = 16, 4, 128
NCORES = 8
HQ = H // NCORES            # 2 query heads per core
ROPE_BASE = 10000.0
SCALE = 1.0 / math.sqrt(HD)
TB = 512                    # t-block (moving free dim) for attention
NT = T // TB                # 4
NCT = C // 128              # 16 contraction tiles for projections
NS = T // 128               # 16 key/value s-tiles
BF16 = mybir.dt.bfloat16
F32 = mybir.dt.float32
EXPF = mybir.ActivationFunctionType.Exp
NEG = -1.0e30

_NC_CACHE = {}


def _bf16(a):
    return np.ascontiguousarray(np.asarray(a, dtype=np.float32).astype(ml_dtypes.bfloat16))


def _emit(tc, dr, out_d):
    nc = tc.nc
    with ExitStack() as ctx:
        def sb(name, bufs):
            return ctx.enter_context(tc.tile_pool(name=name, bufs=bufs))

        def ps(name, bufs):
            return ctx.enter_context(tc.tile_pool(name=name, bufs=bufs, space="PSUM"))

        p_xt = sb("xt", NCT)
        p_wq = sb("wq", NCT)
        p_wo = sb("wo", 1)
        p_trig = sb("trig", 1)
        p_mask = sb("mask", 1)
        p_junk = sb("junk", 1)
        p_qt = sb("qt", HQ)
        p_kt = sb("kt", 1)
        p_v = sb("v", NS)
        p_vt = sb("vt", 2)
        p_yt = sb("yt", HQ)
        p_qraw = sb("qraw", 8)
        p_rtmp = sb("rtmp", 6)
        p_pt = sb("pt", 4)          # [128,1024] bf16 exp strips
        p_acc = sb("acc", 2)        # [128,TB] bf16 denominator partial sums
        p_rb = sb("rb", 2)          # [128,TB] f32 partition_all_reduce out
        p_ri = sb("ri", 2)          # [128,TB] f32 reciprocal out
        p_stage = sb("stage", 4)
        ps_strip = ps("ps_strip", 2)  # [128,1024] score strips (+rope rot)
        ps_acc = ps("ps_acc", 2)      # [128,TB] attention outT accumulators
        ps_cp = ps("ps_cp", 2)        # [128,TB] c_proj / proj k,v chains

        # ---------------- input loads (HWDGE queues: sync + scalar only; the
        # gpsimd queue is software-DGE and burns ~1us of Pool engine time per
        # descriptor, delaying rope adds / evictions) -------------------------
        wqkv = [p_wq.tile([128, 4 * HD], BF16, name=f"wqkv{i}", tag="wq")
                for i in range(NCT)]
        xt = [p_xt.tile([128, T], BF16, name=f"xt{i}", tag="xt") for i in range(NCT)]
        # The 8-chain group (t-blocks 0+1) only reads xt[:, 0:1024]: stream all
        # first-halves at the head of the sync queue (delivery ~1.1us/tile vs
        # ~1.7us/tile consumption), second halves + late tables behind them.
        # wqkv + small tables go on the scalar queue, interleaving with sync on
        # the shared wire.
        nc.sync.dma_start(wqkv[0][:], dr["wqkv"][0:128, :])
        nc.scalar.dma_start(xt[0][:, 0:512], dr["xt"][0:128, 0:512])
        nc.sync.dma_start(xt[0][:, 512:1024], dr["xt"][0:128, 512:1024])
        for i in range(1, NCT):
            nc.sync.dma_start(xt[i][:, 0:1024], dr["xt"][ts(i, 128), 0:1024])
        for i in range(NCT):
            nc.sync.dma_start(xt[i][:, 1024:T], dr["xt"][ts(i, 128), 1024:T])
        for i in range(1, 8):
            nc.scalar.dma_start(wqkv[i][:], dr["wqkv"][ts(i, 128), :])
        wq = wqkv                                    # cols [0, 256) = Wq
        wk = [t[:, 256:384] for t in wqkv]           # cols [256, 384) = Wk
        wv = [t[:, 384:512] for t in wqkv]           # cols [384, 512) = Wv
        # trig first-halves + rot/mask tables land mid-stream (needed by the
        # grp rope evictions at ~31us); wqkv tail keeps pace with the group.
        trig = p_trig.tile([128, 2 * T], BF16, name="trig", tag="trig")
        cost = trig[:, 0:T]
        sint = trig[:, T:2 * T]
        nc.scalar.dma_start(trig[:, 0:1024], dr["trig"][:, 0:1024])
        nc.scalar.dma_start(trig[:, T:T + 1024], dr["trig"][:, T:T + 1024])
        rmat = p_mask.tile([128, 128], BF16, name="rmat", tag="rmat")
        nc.scalar.dma_start(rmat[:], dr["rmat"][:, :])
        masks = p_mask.tile([128, 128], F32, name="masks", tag="mask")
        nc.scalar.dma_start(masks[:], dr["masks"][:, :])
        for i in range(8, NCT):
            nc.scalar.dma_start(wqkv[i][:], dr["wqkv"][ts(i, 128), :])
        nc.scalar.dma_start(trig[:, 1024:T], dr["trig"][:, 1024:T])
        nc.scalar.dma_start(trig[:, T + 1024:2 * T], dr["trig"][:, T + 1024:2 * T])
        wo2 = p_wo.tile([128, HQ * C], BF16, name="wo2", tag="wo")
        wo = [wo2[:, ts(h, C)] for h in range(HQ)]
        nc.scalar.dma_start(wo2[:, 0:C], dr["wo"][:, 0:C])
        nc.scalar.dma_start(wo2[:, C:2 * C], dr["wo"][:, C:2 * C])
        # preload exp table while DMAs run (after the scalar-queue dma issues
        # so LoadActFuncSet doesn't block them)
        junk = p_junk.tile([128, 512], BF16, name="junk", tag="junk")
        nc.vector.memset(junk[0:1, 0:1], 0.0)
        nc.scalar.activation(junk[0:1, 0:1], junk[0:1, 0:1], EXPF, scale=1.0)

        # ---------------- rope helpers (two-phase) ----------------
        def raw_phase(psum):
            """chain psum [128(d), TB] f32 -> raw bf16 SBUF copy (ACT)."""
            raw = p_qraw.tile([128, TB], BF16, name="rraw", tag="qraw")
            nc.scalar.copy(raw[:], psum[:])
            return raw

        def rot_phase(raw, bt, dst):
            """raw [128, TB] bf16 -> RoPE -> dst (bf16 slice [128, TB])."""
            cs = cost[:, ts(bt, TB)]
            sn = sint[:, ts(bt, TB)]
            rot = ps_strip.tile([128, TB], F32, name="rot", tag="strip")
            nc.tensor.matmul(rot[:], lhsT=rmat[:], rhs=raw[:], start=True, stop=True)
            t1 = p_rtmp.tile([128, TB], BF16, name="rt1", tag="rtmp")
            t2 = p_rtmp.tile([128, TB], BF16, name="rt2", tag="rtmp")
            nc.vector.tensor_mul(t1[:], raw[:], cs)
            nc.vector.tensor_mul(t2[:], rot[:], sn)
            nc.gpsimd.tensor_add(dst, t1[:], t2[:])

        qT = [p_qt.tile([128, T], BF16, name=f"qT{h}", tag="qt") for h in range(HQ)]
        kT = p_kt.tile([128, T], BF16, name="kT", tag="kt")
        v = [p_v.tile([128, HD], BF16, name=f"v{s}", tag="v") for s in range(NS)]
        yT = [p_yt.tile([128, T], BF16, name=f"yT{h}", tag="yt") for h in range(HQ)]
        dma_rr = [nc.sync, nc.scalar]

        def v_evict(pv, bt):
            vts = p_vt.tile([128, TB], BF16, name="vts", tag="vt")
            nc.scalar.copy(vts[:], pv[:])
            for j in range(TB // 128):   # sync queue only: keep ACT.SEQ clear
                nc.sync.dma_start_transpose(v[4 * bt + j][:], vts[:, ts(j, 128)])

        # ---------------- c_proj work queue ----------------
        cp_queue = []          # (m, n) items ready to emit
        st_tiles = {}          # m -> stage tile
        evict_rr = [0]
        last_row = [-1]        # set before the finale drain

        def emit_cp_item(finale=False):
            if not cp_queue:
                return False
            m, n = cp_queue.pop(0)
            if n == 0:
                st_tiles[m] = p_stage.tile([128, C], BF16, name="st", tag="stage")
            st = st_tiles[m]
            pc = ps_cp.tile([128, TB], F32, name="pc", tag="cp")
            for h in range(HQ):
                nc.tensor.matmul(
                    pc[:], lhsT=yT[h][:, ts(m, 128)], rhs=wo[h][:, ts(n, TB)],
                    start=(h == 0), stop=(h == HQ - 1))
            e = evict_rr[0] % 2   # Pool/GpSimd cannot read PSUM
            evict_rr[0] += 1
            if e == 0:
                nc.scalar.copy(st[:, ts(n, TB)], pc[:])
            else:
                nc.vector.tensor_copy(st[:, ts(n, TB)], pc[:])
            if m == last_row[0]:   # chunked DMA right after eviction: short tail
                dma_rr[evict_rr[0] % 2].dma_start(out_d[ts(m, 128), ts(n, TB)],
                                                  st[:, ts(n, TB)])
            elif n == 3:
                dma_rr[m % 2].dma_start(out_d[ts(m, 128), :], st[:])
            return True

        def drain_cp(k, finale=False):
            for _ in range(k):
                if not emit_cp_item(finale):
                    return

        # ---------------- attention ----------------
        def attn_block(bt, filler=None):
            """filler: optional callable emitted between strips (PE fill work)."""
            nbs = 4 * (bt + 1)
            for h in range(HQ):
                po = ps_acc.tile([128, TB], F32, name="po", tag="acc")
                acc = p_acc.tile([128, TB], BF16, name="dacc", tag="acc")
                prev = None
                for k in range(nbs // 2):
                    b0, b1 = 2 * k, 2 * k + 1
                    strip = ps_strip.tile([128, 1024], F32, name="sc", tag="strip")
                    pt = p_pt.tile([128, 1024], BF16, name="pt", tag="pt")
                    offs = []
                    for si, bs in enumerate((b0, b1)):
                        j = bs - 4 * bt
                        off = max(j, 0) * 128
                        offs.append(off)
                        if off > 0:   # dead zone: keep the strip-wide exp fed
                            nc.vector.memset(strip[:, ds(512 * si, off)], 0.0)
                        nc.tensor.matmul(
                            strip[:, 512 * si + off:512 * (si + 1)],
                            lhsT=kT[:, ts(bs, 128)],
                            rhs=qT[h][:, ds(bt * TB + off, TB - off)],
                            start=True, stop=True)
                        if j >= 0:   # diagonal: intra-tile causal triangle
                            nc.vector.tensor_add(
                                strip[:, ds(512 * si + off, 128)],
                                strip[:, ds(512 * si + off, 128)],
                                masks[:, :])
                    nc.scalar.activation(pt[:], strip[:], EXPF, scale=SCALE)
                    # denominator partial sums (DVE, partition lanes)
                    if k == 0:
                        nc.vector.tensor_copy(acc[:], pt[:, 0:512])
                    else:
                        nc.vector.tensor_add(
                            acc[:, offs[0]:512], acc[:, offs[0]:512],
                            pt[:, offs[0]:512])
                    nc.vector.tensor_add(
                        acc[:, offs[1]:512], acc[:, offs[1]:512],
                        pt[:, 512 + offs[1]:1024])
                    if filler is not None:
                        filler(h, k)
                    if prev is not None:
                        pv_emit(po, bt, nbs, *prev)
                    prev = (pt, b0, b1, offs)
                pv_emit(po, bt, nbs, *prev)
                # normalize: partition all-reduce -> reciprocal -> fold into evict
                rb = p_rb.tile([128, TB], F32, name="rb", tag="rb")
                nc.gpsimd.partition_all_reduce(rb[:], acc[:], channels=128,
                                               reduce_op=bass_isa.ReduceOp.add)
                ri = p_ri.tile([128, TB], F32, name="ri", tag="ri")
                nc.vector.reciprocal(ri[:], rb[:])
                for e in range(4):   # chunked evict: c_proj(m) starts earlier
                    nc.vector.tensor_mul(
                        yT[h][:, ds(bt * TB + e * 128, 128)],
                        po[:, ts(e, 128)], ri[:, ts(e, 128)])
            cp_queue.extend((4 * bt + sub, n)
                            for sub in range(NT) for n in range(C // TB))

        def pv_emit(po, bt, nbs, pt, b0, b1, offs):
            for si, bs in enumerate((b0, b1)):
                off = offs[si]
                nc.tensor.matmul(
                    po[:, off:TB], lhsT=v[bs][:],
                    rhs=pt[:, 512 * si + off:512 * (si + 1)],
                    start=(bs == 0), stop=(bs == nbs - 1))

        # -------- projections --------
        def proj_chains(bt, tiles):
            """Emit 4 projection chains for t-block bt into psum slices
            tiles = [q0, q1, k, v]."""
            specs = [(wq, 0), (wq, 1), (wk, None), (wv, None)]
            for ci in range(NCT):
                for kind in range(4):
                    w, hh = specs[kind]
                    lhsT = w[ci][:, ts(hh, HD)] if hh is not None else w[ci]
                    nc.tensor.matmul(
                        tiles[kind], lhsT=lhsT, rhs=xt[ci][:, ts(bt, TB)],
                        start=(ci == 0), stop=(ci == NCT - 1))

        # t-blocks 0+1: one ci-interleaved 8-chain group across all 8 PSUM
        # banks — maximizes PE progress while x is still streaming in.
        s0 = ps_strip.tile([128, 1024], F32, name="gs0", tag="strip")
        s1 = ps_strip.tile([128, 1024], F32, name="gs1", tag="strip")
        a0 = ps_acc.tile([128, TB], F32, name="ga0", tag="acc")
        a1 = ps_acc.tile([128, TB], F32, name="ga1", tag="acc")
        c0 = ps_cp.tile([128, TB], F32, name="gc0", tag="cp")
        c1 = ps_cp.tile([128, TB], F32, name="gc1", tag="cp")
        grp = [s0[:, 0:512], s0[:, 512:1024], s1[:, 0:512], s1[:, 512:1024],
               a0[:], a1[:], c0[:], c1[:]]
        specs = [(wq, 0), (wq, 1), (wk, None), (wv, None)]
        for ci in range(NCT):
            for i in range(8):
                b = i // 4
                w, hh = specs[i % 4]
                lhsT = w[ci][:, ts(hh, HD)] if hh is not None else w[ci]
                nc.tensor.matmul(
                    grp[i], lhsT=lhsT, rhs=xt[ci][:, ts(b, TB)],
                    start=(ci == 0), stop=(ci == NCT - 1))
        # raw copies free the chain PSUM quickly (ACT), v straight out
        def grp_raws(b, base):
            return [raw_phase(grp[base + 0]), raw_phase(grp[base + 1]),
                    raw_phase(grp[base + 2]), v_evict(grp[base + 3], b)][:3]

        def rots_for(raws, bt):
            rot_phase(raws[0], bt, qT[0][:, ts(bt, TB)])
            rot_phase(raws[1], bt, qT[1][:, ts(bt, TB)])
            rot_phase(raws[2], bt, kT[:, ts(bt, TB)])

        raws0 = grp_raws(0, 0)
        raws1 = grp_raws(1, 4)

        # -------- t-blocks 2+3 projections run back-to-back on PE while the
        # rope chains of earlier blocks drain on ACT/DVE/Pool ------------------
        def proj_block_late(bt, pending_rots):
            sq = ps_strip.tile([128, 1024], F32, name="ps", tag="strip")
            ck = ps_cp.tile([128, TB], F32, name="pk", tag="cp")
            cv = ps_cp.tile([128, TB], F32, name="pv", tag="cp")
            tiles = [sq[:, 0:512], sq[:, 512:1024], ck[:], cv[:]]
            specs = [(wq, 0), (wq, 1), (wk, None), (wv, None)]
            for ci in range(NCT):
                for kind in range(4):
                    w, hh = specs[kind]
                    lhsT = w[ci][:, ts(hh, HD)] if hh is not None else w[ci]
                    nc.tensor.matmul(
                        tiles[kind], lhsT=lhsT, rhs=xt[ci][:, ts(bt, TB)],
                        start=(ci == 0), stop=(ci == NCT - 1))
                if ci == 0:
                    for rr, rbt in pending_rots:
                        rots_for(rr, rbt)
            rq0 = raw_phase(sq[:, 0:512])
            rq1 = raw_phase(sq[:, 512:1024])
            rk = raw_phase(ck[:])
            v_evict(cv, bt)
            return [rq0, rq1, rk]

        raws2 = proj_block_late(2, [(raws0, 0), (raws1, 1)])
        raws3 = proj_block_late(3, [(raws2, 2)])
        rots_for(raws3, 3)

        def filler(h, k):
            drain_cp(2 if len(cp_queue) > 12 else 1)

        # block order 2,3,1,0: the un-fillable first block is mid-sized, and
        # the big blocks (3 especially) run with a loaded c_proj queue.
        attn_block(0)
        attn_block(1, filler=filler)
        attn_block(2, filler=filler)
        attn_block(3, filler=filler)
        last_row[0] = cp_queue[-1][0]
        drain_cp(len(cp_queue), finale=True)


def build_nc():
    if "nc" in _NC_CACHE:
        return _NC_CACHE["nc"]
    nc = bacc.Bacc("TRN2", target_bir_lowering=False, debug=False, num_devices=NCORES)
    dr = {}

    def din(name, shape, dt):
        dr[name] = nc.dram_tensor(name, shape, dt, kind="ExternalInput").ap()

    din("xt", (C, T), BF16)
    din("wqkv", (C, 4 * HD), BF16)
    din("wo", (HD, HQ * C), BF16)
    din("trig", (HD, 2 * T), BF16)
    din("masks", (128, 128), F32)
    din("rmat", (HD, HD), BF16)
    out_d = nc.dram_tensor("out", (T, C), BF16, kind="ExternalOutput").ap()

    with tile.TileContext(nc) as tc:
        _emit(tc, dr, out_d)
    nc.compile()
    _NC_CACHE["nc"] = nc
    return nc


def make_in_maps(x, Wq, Wk, Wv, Wo, position_ids):
    """Host-side sharding + constant tables. Returns one input dict per core."""
    x = np.asarray(x, dtype=np.float32)
    xt = _bf16(x.reshape(T, C).T)                      # [C, T]

    pos = np.asarray(position_ids).astype(np.float64)  # [T]
    inv = 1.0 / (ROPE_BASE ** (np.arange(0, HD, 2, dtype=np.float64) / HD))
    fr = pos[:, None] * inv[None, :]                   # [T, 64]
    emb = np.concatenate([fr, fr], axis=-1)            # [T, 128]
    cost = _bf16(np.cos(emb).T)                        # [128, T]
    sint = _bf16(np.sin(emb).T)

    si = np.arange(128)[:, None]
    ti = np.arange(128)[None, :]
    masks = np.where(si > ti, NEG, 0.0).astype(np.float32)   # [128, 128] triangle

    # rotate_half operator: rot = R @ q  with  rot[d<64] = -q[d+64],
    # rot[d>=64] = q[d-64].  matmul computes lhsT.T @ rhs, so ship R.T.
    R = np.zeros((HD, HD), dtype=np.float32)
    R[np.arange(64), np.arange(64) + 64] = -1.0
    R[np.arange(64, 128), np.arange(64, 128) - 64] = 1.0
    rmat = _bf16(R.T)

    Wq = np.asarray(Wq, dtype=np.float32)
    Wk = np.asarray(Wk, dtype=np.float32)
    Wv = np.asarray(Wv, dtype=np.float32)
    Wo = np.asarray(Wo, dtype=np.float32)

    in_maps = []
    for i in range(NCORES):
        g = i // (NCORES // KH)                        # kv head for this core
        in_maps.append({
            "xt": xt,
            "wqkv": _bf16(np.concatenate([
                Wq[:, i * HQ * HD:(i + 1) * HQ * HD],
                Wk[:, g * HD:(g + 1) * HD],
                Wv[:, g * HD:(g + 1) * HD]], axis=1)),
            "wo": _bf16(np.concatenate([
                Wo[i * HQ * HD + h * HD:i * HQ * HD + (h + 1) * HD, :]
                for h in range(HQ)], axis=1)),
            "trig": np.concatenate([cost, sint], axis=1),
            "masks": masks,
            "rmat": rmat,
        })
    return in_maps


def run(inputs, trace=False):
    nc = build_nc()
    in_maps = make_in_maps(**inputs)
    res = run_bass_kernel_spmd(
        nc, in_maps, core_ids=list(range(NCORES)), trace=trace)
    out = np.zeros((T, C), dtype=np.float32)
    for i in range(NCORES):
        out += np.asarray(res.results[i]["out"], dtype=np.float32)
    return out.reshape(B, T, C), res


def kernel(x, Wq, Wk, Wv, Wo, position_ids):
    out, _ = run(dict(x=x, Wq=Wq, Wk=Wk, Wv=Wv, Wo=Wo,
                      position_ids=position_ids), trace=False)
    return out
